# revision 1
# baseline (speedup 1.0000x reference)
"""Expert-parallel MoE FFN for Trainium2 — one expert per NeuronCore (8 cores).

Strategy
--------
The reference computes, per token, the sum of top-2 expert FFN outputs (binary
combine mask, no gate weighting).  We shard along the expert axis: core ``e``
holds expert ``e``'s weights (W1[e], b1[e], W2[e], b2[e]) and processes only
the tokens that routed to it.

Host side (cheap, O(T*D*E) = 34 MFLOP):
  * gating softmax + top-2 (replicates jax.nn.softmax + jax.lax.top_k
    tie-breaking exactly: stable argsort on the fp32 scores, descending),
  * gather each expert's tokens, pad to a uniform capacity (all cores run the
    same NEFF), and pre-transpose to [D, cap] so the device needs no
    transposes at all,
  * scatter-add the 8 per-expert outputs back into the [T, D] result.

Device side (the heavy part, ~19 GFLOP/core):
  hT = relu(W1^T-chained matmuls + b1);  yT = W2-chained matmuls + b2,
  everything kept in "transposed" layout: contraction dims live on SBUF
  partitions for both layers, so mm1's output feeds mm2 directly.
  bf16 inputs, fp32 PSUM accumulation.
"""

import numpy as np
import ml_dtypes

import concourse.bacc as bacc
import concourse.mybir as mybir
import concourse.tile as tile
from concourse.bass_utils import run_bass_kernel_spmd
from concourse._compat import get_trn_type

D_MODEL = 1024
D_FF = 4096
N_EXP = 8
TOP_K = 2
TT = 384  # token tile: matmul free dim; 384 fp32 = 1536 B fits one PSUM bank
KD = D_MODEL // 128  # 8 contraction chunks over d_model
KF = D_FF // 128  # 32 contraction chunks over d_ff

BF16 = mybir.dt.bfloat16
F32 = mybir.dt.float32

_programs: dict[int, object] = {}


def _build_program(cap: int):
    """Bass/Tile program: [D,cap] tokens -> two-layer FFN -> [D,cap] output."""
    assert cap % TT == 0
    nt = cap // TT
    nc = bacc.Bacc(get_trn_type() or "TRN2", target_bir_lowering=False, debug=False)

    xT_d = nc.dram_tensor("xT", [D_MODEL, cap], BF16, kind="ExternalInput").ap()
    w1_d = nc.dram_tensor("W1", [D_MODEL, D_FF], BF16, kind="ExternalInput").ap()
    b1_d = nc.dram_tensor("b1", [128, KF], F32, kind="ExternalInput").ap()
    w2_d = nc.dram_tensor("W2", [D_FF, D_MODEL], BF16, kind="ExternalInput").ap()
    b2_d = nc.dram_tensor("b2", [128, KD], F32, kind="ExternalInput").ap()
    y_d = nc.dram_tensor("yT", [D_MODEL, cap], F32, kind="ExternalOutput").ap()

    xT_t = xT_d.rearrange("(k p) c -> k p c", p=128)
    w1_t = w1_d.rearrange("(k p) f -> k p f", p=128)
    w2_t = w2_d.rearrange("(f p) m -> f p m", p=128)
    y_t = y_d.rearrange("(m p) c -> m p c", p=128)

    # SBUF budget per partition (cap=1152): x 18K + W1 64K + W2 64K (bf16)
    # + h 30K + y 6K + biases ~0.2K  ->  ~182K of 192K.
    h_bufs = 40 if cap <= 1536 else KF + 2

    with tile.TileContext(nc) as tc:
        with (
            tc.tile_pool(name="sb", bufs=1) as sb,
            tc.tile_pool(name="hp", bufs=h_bufs) as hp,
            tc.tile_pool(name="yp", bufs=4) as yp,
            tc.tile_pool(name="pp1", bufs=4, space="PSUM") as pp1,
            tc.tile_pool(name="pp2", bufs=4, space="PSUM") as pp2,
        ):
            b1_sb = sb.tile([128, KF], F32, tag="b1", name="b1_sb")
            nc.sync.dma_start(b1_sb[:], b1_d)
            b2_sb = sb.tile([128, KD], F32, tag="b2", name="b2_sb")
            nc.sync.dma_start(b2_sb[:], b2_d)

            x_sb = []
            for k in range(KD):
                t = sb.tile([128, cap], BF16, tag=f"x{k}", name=f"x_sb{k}")
                nc.sync.dma_start(t[:], xT_t[k])
                x_sb.append(t)

            # W1 arrives in 4 column groups so mm1 can start after group 0.
            w1_sb = [
                sb.tile([128, D_FF], BF16, tag=f"w1_{k}", name=f"w1_sb{k}")
                for k in range(KD)
            ]
            for g in range(4):
                gs = slice(g * (D_FF // 4), (g + 1) * (D_FF // 4))
                for k in range(KD):
                    nc.sync.dma_start(w1_sb[k][:, gs], w1_t[k][:, gs])

            w2_sb = []
            for f in range(KF):
                t = sb.tile([128, D_MODEL], BF16, tag=f"w2_{f}", name=f"w2_sb{f}")
                nc.sync.dma_start(t[:], w2_t[f])
                w2_sb.append(t)

            for it in range(nt):
                tsl = slice(it * TT, (it + 1) * TT)

                # mm1: hT[f*128+p, t] = relu(sum_d W1[d, f*128+p] * xT[d, t] + b1)
                h_tiles = []
                for f in range(KF):
                    ps = pp1.tile([128, TT], F32, tag="ps1", name=f"ps1_{it}_{f}")
                    for k in range(KD):
                        nc.tensor.matmul(
                            ps[:],
                            w1_sb[k][:, f * 128 : (f + 1) * 128],
                            x_sb[k][:, tsl],
                            start=(k == 0),
                            stop=(k == KD - 1),
                        )
                    ht = hp.tile([128, TT], BF16, tag="h", name=f"h_{it}_{f}")
                    nc.scalar.activation(
                        ht[:],
                        ps[:],
                        mybir.ActivationFunctionType.Relu,
                        bias=b1_sb[:, f : f + 1],
                    )
                    h_tiles.append(ht)

                # mm2: yT[m*128+p, t] = sum_f W2[f, m*128+p] * hT[f, t] + b2
                for m in range(KD):
                    ps2 = pp2.tile([128, TT], F32, tag="ps2", name=f"ps2_{it}_{m}")
                    for f in range(KF):
                        nc.tensor.matmul(
                            ps2[:],
                            w2_sb[f][:, m * 128 : (m + 1) * 128],
                            h_tiles[f][:],
                            start=(f == 0),
                            stop=(f == KF - 1),
                        )
                    yt = yp.tile([128, TT], F32, tag="y", name=f"y_{it}_{m}")
                    nc.vector.tensor_scalar_add(yt[:], ps2[:], b2_sb[:, m : m + 1])
                    nc.sync.dma_start(y_t[m][:, tsl], yt[:])

    nc.compile()
    return nc


def _gating_topk(x, Wg, bg):
    """Replicates jax.nn.softmax + jax.lax.top_k(..., 2) in fp32 numpy."""
    logits = x @ Wg + bg
    m = logits.max(axis=1, keepdims=True)
    e = np.exp(logits - m)
    scores = e / e.sum(axis=1, keepdims=True)
    # top_k: descending, ties broken toward the lower index (stable).
    order = np.argsort(-scores, axis=1, kind="stable")
    return order[:, :TOP_K]


def _prepare(x, Wg, bg, W1, b1, W2, b2):
    x = np.ascontiguousarray(np.asarray(x, dtype=np.float32))
    topk = _gating_topk(x, np.asarray(Wg, np.float32), np.asarray(bg, np.float32))
    idx = [np.nonzero((topk == e).any(axis=1))[0] for e in range(N_EXP)]
    counts = [len(i) for i in idx]
    cap = max(TT, -(-max(counts) // TT) * TT)

    bf16 = ml_dtypes.bfloat16
    in_maps = []
    for e in range(N_EXP):
        xg = np.zeros((cap, D_MODEL), np.float32)
        xg[: counts[e]] = x[idx[e]]
        in_maps.append(
            {
                "xT": np.ascontiguousarray(xg.T).astype(bf16),
                "W1": np.asarray(W1[e], np.float32).astype(bf16),
                "b1": np.ascontiguousarray(
                    np.asarray(b1[e], np.float32).reshape(KF, 128).T
                ),
                "W2": np.asarray(W2[e], np.float32).astype(bf16),
                "b2": np.ascontiguousarray(
                    np.asarray(b2[e], np.float32).reshape(KD, 128).T
                ),
            }
        )
    return x, idx, counts, cap, in_maps


def _run(x, Wg, bg, W1, b1, W2, b2, **run_kwargs):
    x, idx, counts, cap, in_maps = _prepare(x, Wg, bg, W1, b1, W2, b2)
    prog = _programs.get(cap)
    if prog is None:
        prog = _programs.setdefault(cap, _build_program(cap))
    res = run_bass_kernel_spmd(
        prog, in_maps, core_ids=list(range(N_EXP)), **run_kwargs
    )
    out = np.zeros_like(x)
    for e in range(N_EXP):
        ye = np.asarray(res.results[e]["yT"], np.float32)[:, : counts[e]].T
        out[idx[e]] += ye
    return out, res


def kernel(x, Wg, bg, W1, b1, W2, b2):
    out, _ = _run(x, Wg, bg, W1, b1, W2, b2)
    return out


# revision 2
# speedup vs baseline: 1.0062x; 1.0062x over previous
"""Expert-parallel MoE FFN for Trainium2 — one expert per NeuronCore (8 cores).

Strategy
--------
The reference computes, per token, the sum of top-2 expert FFN outputs (binary
combine mask, no gate weighting).  We shard along the expert axis: core ``e``
holds expert ``e``'s weights (W1[e], b1[e], W2[e], b2[e]) and processes only
the tokens that routed to it.

Host side (cheap, O(T*D*E) = 34 MFLOP):
  * gating softmax + top-2 (replicates jax.nn.softmax + jax.lax.top_k
    tie-breaking exactly: stable argsort on the fp32 scores, descending),
  * gather each expert's tokens, pad to a uniform capacity (all cores run the
    same NEFF), and pre-transpose to [D, cap] so the device needs no
    transposes at all,
  * scatter-add the 8 per-expert outputs back into the [T, D] result.

Device side (the heavy part, ~18 GFLOP/core):
  hT = relu(W1^T-chained matmuls + b1);  yT = W2-chained matmuls + b2,
  everything kept in "transposed" layout: contraction dims live on SBUF
  partitions for both layers, so mm1's output feeds mm2 directly.
  bf16 inputs, fp32 PSUM accumulation.

DMA design: one trigger per group (a trigger costs ~600 ns on its queue
engine), groups ordered so the first token-tile's operands land first, and
W1 rides the GpSimd queue in parallel with x on the Sync queue.  Each DMA
group is its own SBUF tensor so Tile's whole-tile deps are exact.
"""

import numpy as np
import ml_dtypes

import concourse.bacc as bacc
import concourse.mybir as mybir
import concourse.tile as tile
from concourse.bass_utils import run_bass_kernel_spmd
from concourse._compat import get_trn_type

D_MODEL = 1024
D_FF = 4096
N_EXP = 8
TOP_K = 2
KD = D_MODEL // 128  # 8 contraction chunks over d_model
KF = D_FF // 128  # 32 contraction chunks over d_ff

BF16 = mybir.dt.bfloat16
F32 = mybir.dt.float32

_programs: dict[tuple, object] = {}


def _split_points(n, parts):
    """Split range(n) into `parts` contiguous chunks of near-equal size."""
    bounds = [round(i * n / parts) for i in range(parts + 1)]
    return list(zip(bounds[:-1], bounds[1:]))


def _build_program(cap: int, tt: int):
    """Bass/Tile program: [D,cap] tokens -> two-layer FFN -> [D,cap] output."""
    assert cap % tt == 0
    nt = cap // tt
    nc = bacc.Bacc(get_trn_type() or "TRN2", target_bir_lowering=False, debug=False)

    xT_d = nc.dram_tensor("xT", [D_MODEL, cap], BF16, kind="ExternalInput").ap()
    w1_d = nc.dram_tensor("W1", [D_MODEL, D_FF], BF16, kind="ExternalInput").ap()
    b1_d = nc.dram_tensor("b1", [128, KF], F32, kind="ExternalInput").ap()
    w2_d = nc.dram_tensor("W2", [D_FF, D_MODEL], BF16, kind="ExternalInput").ap()
    b2_d = nc.dram_tensor("b2", [128, KD], F32, kind="ExternalInput").ap()
    y_d = nc.dram_tensor("yT", [D_MODEL, cap], F32, kind="ExternalOutput").ap()

    # DRAM views with the partition index innermost: [128, chunk, free]
    xT_v = xT_d.rearrange("(k p) c -> p k c", p=128)
    w1_v = w1_d.rearrange("(k p) f -> p k f", p=128)
    w2_v = w2_d.rearrange("(f p) m -> p f m", p=128)
    y_v = y_d.rearrange("(m p) c -> m p c", p=128)

    # W1 column groups: first group small so mm1 can start early.
    W1_GROUPS = [(0, 512), (512, 2048), (2048, D_FF)]
    # W2 f-chunk groups (mm2 accumulates f ascending).
    W2_GROUPS = [(0, KF // 2), (KF // 2, KF)]

    with tile.TileContext(nc) as tc:
        with (
            tc.tile_pool(name="sb", bufs=1) as sb,
            tc.tile_pool(name="hp", bufs=40) as hp,
            tc.tile_pool(name="yp", bufs=4) as yp,
            tc.tile_pool(name="pp1", bufs=4, space="PSUM") as pp1,
            tc.tile_pool(name="pp2", bufs=4, space="PSUM") as pp2,
        ):
            # ---- inputs: one DMA trigger per group -----------------------
            # x: token-tile 0 first (Sync queue).
            x_gs = []  # one sbuf tensor per token-tile group
            xa = sb.tile([128, KD * tt], BF16, tag="xa", name="xa")
            nc.sync.dma_start(
                xa.rearrange("p (k c) -> p k c", c=tt), xT_v[:, :, 0:tt]
            )
            x_gs.append(xa)
            if nt > 1:
                rest = cap - tt
                xb = sb.tile([128, KD * rest], BF16, tag="xb", name="xb")
                nc.sync.dma_start(
                    xb.rearrange("p (k c) -> p k c", c=rest), xT_v[:, :, tt:cap]
                )
                x_gs.append(xb)

            def x_rhs(k, it):
                if it == 0:
                    return x_gs[0][:, k * tt : (k + 1) * tt]
                rest = cap - tt
                lo = k * rest + (it - 1) * tt
                return x_gs[1][:, lo : lo + tt]

            # W1 groups (GpSimd queue, parallel with x on Sync).
            w1_gs = []
            for gi, (lo, hi) in enumerate(W1_GROUPS):
                w = hi - lo
                t = sb.tile([128, KD * w], BF16, tag=f"w1g{gi}", name=f"w1g{gi}")
                nc.gpsimd.dma_start(
                    t.rearrange("p (k f) -> p k f", f=w), w1_v[:, :, lo:hi]
                )
                w1_gs.append((lo, hi, t))

            def w1_lhsT(k, f):
                col = f * 128
                for lo, hi, t in w1_gs:
                    if lo <= col < hi:
                        w = hi - lo
                        base = k * w + (col - lo)
                        return t[:, base : base + 128]
                raise AssertionError

            b1_sb = sb.tile([128, KF], F32, tag="b1", name="b1_sb")
            nc.sync.dma_start(b1_sb[:], b1_d)
            b2_sb = sb.tile([128, KD], F32, tag="b2", name="b2_sb")
            nc.sync.dma_start(b2_sb[:], b2_d)

            # W2 groups (GpSimd queue, after W1).
            w2_gs = []
            for gi, (flo, fhi) in enumerate(W2_GROUPS):
                nf = fhi - flo
                t = sb.tile([128, nf * D_MODEL], BF16, tag=f"w2g{gi}", name=f"w2g{gi}")
                nc.gpsimd.dma_start(
                    t.rearrange("p (f m) -> p f m", m=D_MODEL), w2_v[:, flo:fhi, :]
                )
                w2_gs.append((flo, fhi, t))

            def w2_lhsT(f, m):
                for flo, fhi, t in w2_gs:
                    if flo <= f < fhi:
                        base = (f - flo) * D_MODEL + m * 128
                        return t[:, base : base + 128]
                raise AssertionError

            # ---- compute --------------------------------------------------
            for it in range(nt):
                tsl = slice(it * tt, (it + 1) * tt)

                # mm1: hT[f*128+p, t] = relu(sum_d W1[d, f*128+p]*xT[d, t] + b1)
                h_tiles = []
                for f in range(KF):
                    ps = pp1.tile([128, tt], F32, tag="ps1", name=f"ps1_{it}_{f}")
                    for k in range(KD):
                        nc.tensor.matmul(
                            ps[:],
                            w1_lhsT(k, f),
                            x_rhs(k, it),
                            start=(k == 0),
                            stop=(k == KD - 1),
                        )
                    ht = hp.tile([128, tt], BF16, tag="h", name=f"h_{it}_{f}")
                    nc.scalar.activation(
                        ht[:],
                        ps[:],
                        mybir.ActivationFunctionType.Relu,
                        bias=b1_sb[:, f : f + 1],
                    )
                    h_tiles.append(ht)

                # mm2: yT[m*128+p, t] = sum_f W2[f, m*128+p] * hT[f, t] + b2
                for m in range(KD):
                    ps2 = pp2.tile([128, tt], F32, tag="ps2", name=f"ps2_{it}_{m}")
                    for f in range(KF):
                        nc.tensor.matmul(
                            ps2[:],
                            w2_lhsT(f, m),
                            h_tiles[f][:],
                            start=(f == 0),
                            stop=(f == KF - 1),
                        )
                    yt = yp.tile([128, tt], F32, tag="y", name=f"y_{it}_{m}")
                    nc.vector.tensor_scalar_add(yt[:], ps2[:], b2_sb[:, m : m + 1])
                    nc.sync.dma_start(y_v[m][:, tsl], yt[:])

    nc.compile()
    return nc


def _gating_topk(x, Wg, bg):
    """Replicates jax.nn.softmax + jax.lax.top_k(..., 2) in fp32 numpy."""
    logits = x @ Wg + bg
    m = logits.max(axis=1, keepdims=True)
    e = np.exp(logits - m)
    scores = e / e.sum(axis=1, keepdims=True)
    # top_k: descending, ties broken toward the lower index (stable).
    order = np.argsort(-scores, axis=1, kind="stable")
    return order[:, :TOP_K]


def _capacity(max_count):
    nt = max(1, -(-max_count // 512))  # token tiles (PSUM bank: <=512 fp32)
    tt = -(-max_count // nt)
    tt = -(-tt // 4) * 4  # multiple of 4 for aligned fp32 rows
    return nt * tt, tt


def _prepare(x, Wg, bg, W1, b1, W2, b2):
    x = np.ascontiguousarray(np.asarray(x, dtype=np.float32))
    topk = _gating_topk(x, np.asarray(Wg, np.float32), np.asarray(bg, np.float32))
    idx = [np.nonzero((topk == e).any(axis=1))[0] for e in range(N_EXP)]
    counts = [len(i) for i in idx]
    cap, tt = _capacity(max(counts))

    bf16 = ml_dtypes.bfloat16
    in_maps = []
    for e in range(N_EXP):
        xg = np.zeros((cap, D_MODEL), np.float32)
        xg[: counts[e]] = x[idx[e]]
        in_maps.append(
            {
                "xT": np.ascontiguousarray(xg.T).astype(bf16),
                "W1": np.asarray(W1[e], np.float32).astype(bf16),
                "b1": np.ascontiguousarray(
                    np.asarray(b1[e], np.float32).reshape(KF, 128).T
                ),
                "W2": np.asarray(W2[e], np.float32).astype(bf16),
                "b2": np.ascontiguousarray(
                    np.asarray(b2[e], np.float32).reshape(KD, 128).T
                ),
            }
        )
    return x, idx, counts, cap, tt, in_maps


def _run(x, Wg, bg, W1, b1, W2, b2, **run_kwargs):
    x, idx, counts, cap, tt, in_maps = _prepare(x, Wg, bg, W1, b1, W2, b2)
    key = (cap, tt)
    prog = _programs.get(key)
    if prog is None:
        prog = _programs.setdefault(key, _build_program(cap, tt))
    res = run_bass_kernel_spmd(
        prog, in_maps, core_ids=list(range(N_EXP)), **run_kwargs
    )
    out = np.zeros_like(x)
    for e in range(N_EXP):
        ye = np.asarray(res.results[e]["yT"], np.float32)[:, : counts[e]].T
        out[idx[e]] += ye
    return out, res


def kernel(x, Wg, bg, W1, b1, W2, b2):
    out, _ = _run(x, Wg, bg, W1, b1, W2, b2)
    return out
